# revision 3
# baseline (speedup 1.0000x reference)
"""Trainium2 Bass kernel for nn_DecoderRNN (Bahdanau-attention GRU decoder).

Key algebraic simplification (verified exact vs the reference): the attention
MLP has no nonlinearities, so the pre-softmax score is
    score[b,s] = enc[b,s,:].(W1_enc^T u) + (h-dependent terms constant in s),
with u = W2^T W3^T v_att.  Softmax over s is shift-invariant, so the attention
weights are independent of the hidden state and of t.  The whole attention
collapses to a one-time precompute of a per-batch context vector; the rest is
one GRU recurrence plus one large output projection.

Distribution across the 8 cores: the recurrence (small, weight-streaming
bound, doesn't shard) is replicated; the vocab projection W_out (the dominant
cost) is sharded column-wise, 4000 vocab columns per core.  Full inputs are
taken on the host, per-core input maps are built, and the 8 logits slices are
gathered and reassembled.

Precision (chosen so that argmax(logits) matches an fp32 reference exactly on
this problem's data): fp16 matmuls for the recurrence and embedding input
path, fp32 for softmax/context/gates, and a 3-pass bf16 hi/lo split
(bf16x3 pseudo-fp32) for the vocab projection.
"""

import sys

sys.path.insert(0, "/opt/trn_rl_repo")

from contextlib import ExitStack

import numpy as np

import concourse.bacc as bacc
import concourse.mybir as mybir
from concourse import masks
from concourse.tile import TileContext
from concourse.bass_utils import run_bass_kernel_spmd

F32 = mybir.dt.float32
F16 = mybir.dt.float16
BF16 = mybir.dt.bfloat16
AF = mybir.ActivationFunctionType
OP = mybir.AluOpType
AX = mybir.AxisListType

B, S, H, E, T, V = 64, 80, 512, 512, 30, 32000
NCORE = 8
VL = V // NCORE            # 4000 vocab cols per core
NT = T - 1                 # 29 decode steps
TB = NT * B                # 1856 (t-major row index)
NK = H // 128              # 4 contraction chunks over H (= over E)
G3 = 3 * H                 # 1536 gate dim
NGRP = (NT + 1) // 2       # 15 projection groups of <=2 steps
VT = 500                   # vocab tile (one psum bank holds 512 f32)
NVT = VL // VT             # 8
WAVES = [(0, 1, 2), (3, 4, 5), (6, 7)]
BS = B * S


def _emit(nc, tc, dram):
    (encT, xsT, w_hhT, w_ih_eT, w_ih_cT, w1e, w2, w3, v4, b_ih, b_hh,
     w_outT, logits) = dram

    ctx = ExitStack()
    with ctx:
        pers = ctx.enter_context(tc.tile_pool(name="pers", bufs=1))
        hpool = ctx.enter_context(tc.tile_pool(name="h", bufs=2))
        gates = ctx.enter_context(tc.tile_pool(name="gates", bufs=1))
        htp = ctx.enter_context(tc.tile_pool(name="htp", bufs=2))
        evac = ctx.enter_context(tc.tile_pool(name="evac", bufs=4))
        dscr = ctx.enter_context(tc.tile_pool(name="dscr", bufs=1,
                                              space="DRAM"))

        ident = pers.tile([128, 128], F32)
        masks.make_identity(nc, ident[:])
        ones = pers.tile([128, 1], F32)
        nc.vector.memset(ones[:], 1.0)

        gicb = pers.tile([64, G3], F32)      # gi_ctx + b_ih (+ b_hh on r,z)
        bhh_n = pers.tile([64, H], F32)      # b_hh n-part broadcast
        whh16 = pers.tile([128, NK, G3], F16)
        wihe16 = pers.tile([128, NK, G3], F16)
        xs16 = pers.tile([128, NK, NT, B], F16)
        hTh = pers.tile([128, NK, NT, B], BF16)
        hTl = pers.tile([128, NK, NT, B], BF16)
        ctxT = pers.tile([128, NK, B], F32)
        wvec = pers.tile([128, NK], F32)
        hT16_0 = pers.tile([128, NK, B], F16)

        e_scr = dscr.tile([1, BS], F32)
        rs_scr = dscr.tile([1, B], F32)

        # ---------------- phase A/B: attention precompute ----------------
        with tc.tile_pool(name="p0", bufs=1) as p0, \
             tc.tile_pool(name="p0s", bufs=2) as p0s, \
             tc.tile_pool(name="p0e", bufs=1) as p0e, \
             tc.tile_pool(name="ps0", bufs=1, space="PSUM") as ps0:
            # v_att -> [128, 4] (chunk-major)
            vsb = p0.tile([128, NK], F32)
            nc.sync.dma_start(vsb[:], v4.ap().rearrange("c p -> p c"))

            # matvec chain w = W1_enc^T (W2^T (W3^T v_att)), fp32 on PE
            src = vsb
            for wdram, last in ((w3, False), (w2, False), (w1e, True)):
                wmat = p0s.tile([128, NK, H], F32, tag="wmat")
                for c in range(NK):
                    nc.sync.dma_start(
                        wmat[:, c, :], wdram.ap()[128 * c:128 * (c + 1), :])
                mv = ps0.tile([128, NK], F32, tag="mv")
                for j in range(NK):
                    for c in range(NK):
                        nc.tensor.matmul(
                            mv[:, j:j + 1], wmat[:, c, 128 * j:128 * (j + 1)],
                            src[:, c:c + 1], start=(c == 0), stop=(c == NK - 1))
                dst = wvec if last else p0s.tile([128, NK], F32, tag="mvout")
                nc.vector.tensor_copy(dst[:], mv[:])
                src = dst

            # scores[b,s] = sum_f encT[f,bs] w[f]: DVE per-chunk partials in
            # acc, then PE ones-matmul partition-reduce; exp on ACT; e goes to
            # DRAM so it can be broadcast-DMA'd back across partitions.
            acc = p0.tile([128, BS], F32)
            for c in range(NK):
                encc = p0e.tile([128, BS], F32, tag="encc")
                nc.sync.dma_start(encc[:], encT.ap()[128 * c:128 * (c + 1), :])
                if c == 0:
                    nc.vector.tensor_scalar_mul(acc[:], encc[:], wvec[:, 0:1])
                else:
                    nc.vector.scalar_tensor_tensor(
                        acc[:], encc[:], wvec[:, c:c + 1], acc[:],
                        op0=OP.mult, op1=OP.add)
            for j in range(BS // 512):
                sc = ps0.tile([1, 512], F32, tag="sc")
                nc.tensor.matmul(sc[:], ones[:], acc[:, 512 * j:512 * (j + 1)],
                                 start=True, stop=True)
                ej = p0s.tile([1, 512], F32, tag="ej")
                nc.scalar.activation(ej[:], sc[:], AF.Exp)
                nc.sync.dma_start(e_scr[:, 512 * j:512 * (j + 1)], ej[:])
            ebc = p0.tile([128, BS], F32)
            nc.gpsimd.dma_start(ebc[:], e_scr[:].broadcast_to([128, BS]))

            # per-batch 1/sum_s e, broadcast across partitions the same way
            rs = p0s.tile([1, B], F32, tag="rs")
            nc.vector.tensor_reduce(
                rs[:], ebc[0:1, :].rearrange("p (b s) -> p b s", s=S),
                axis=AX.X, op=OP.add)
            rsr = p0s.tile([1, B], F32, tag="rsr")
            nc.vector.reciprocal(rsr[:], rs[:])
            nc.sync.dma_start(rs_scr[:], rsr[:])
            rsb = p0.tile([128, B], F32)
            nc.gpsimd.dma_start(rsb[:], rs_scr[:].broadcast_to([128, B]))

            # context^T[f, b] = (sum_s e[b,s] enc[b,s,f]) / rs[b]
            for c in range(NK):
                encc = p0e.tile([128, BS], F32, tag="encc")
                nc.sync.dma_start(encc[:], encT.ap()[128 * c:128 * (c + 1), :])
                nc.vector.tensor_mul(acc[:], encc[:], ebc[:])
                ctxu = p0s.tile([128, B], F32, tag="ctxu")
                nc.vector.tensor_reduce(
                    ctxu[:], acc[:].rearrange("p (b s) -> p b s", s=S),
                    axis=AX.X, op=OP.add)
                nc.vector.tensor_mul(ctxT[:, c, :], ctxu[:], rsb[:])

        # ---------------- phase C: gi_ctx + bias folding ----------------
        with tc.tile_pool(name="p1", bufs=1) as p1, \
             tc.tile_pool(name="ps1", bufs=1, space="PSUM") as ps1:
            wihc = p1.tile([128, NK, G3], F32)
            for c in range(NK):
                nc.sync.dma_start(
                    wihc[:, c, :], w_ih_cT.ap()[128 * c:128 * (c + 1), :])
            gic = ps1.tile([64, G3], F32)
            for c in range(NK):
                for nn in range(3):
                    nc.tensor.matmul(
                        gic[:, 512 * nn:512 * (nn + 1)], ctxT[:, c, :],
                        wihc[:, c, 512 * nn:512 * (nn + 1)],
                        start=(c == 0), stop=(c == NK - 1))
            bihbc = p1.tile([64, G3], F32)
            nc.gpsimd.dma_start(bihbc[:], b_ih.ap().broadcast_to([64, G3]))
            bhhbc = p1.tile([64, G3], F32)
            nc.gpsimd.dma_start(bhhbc[:], b_hh.ap().broadcast_to([64, G3]))
            brz = p1.tile([64, 2 * H], F32)
            nc.vector.tensor_add(brz[:], bihbc[:, 0:2 * H], bhhbc[:, 0:2 * H])
            nc.vector.tensor_add(gicb[:, 0:2 * H], gic[:, 0:2 * H], brz[:])
            nc.vector.tensor_add(gicb[:, 2 * H:], gic[:, 2 * H:],
                                 bihbc[:, 2 * H:])
            nc.vector.tensor_copy(bhh_n[:], bhhbc[:, 2 * H:])

        # ---------------- phase D: weight conversion --------------------
        wout = ctx.enter_context(tc.tile_pool(name="wout", bufs=1))
        wh = wout.tile([128, NK, VL], BF16)
        wl = wout.tile([128, NK, VL], BF16)
        with tc.tile_pool(name="wst", bufs=2) as wst:
            HV = VL // 2
            for c in range(NK):
                for hh in range(2):
                    st = wst.tile([128, HV], F32, tag="st")
                    sl = slice(HV * hh, HV * (hh + 1))
                    nc.sync.dma_start(
                        st[:], w_outT.ap()[128 * c:128 * (c + 1), sl])
                    nc.vector.tensor_copy(wh[:, c, sl], st[:])
                    nc.vector.tensor_sub(wl[:, c, sl], st[:], wh[:, c, sl])
            for c in range(NK):
                st = wst.tile([128, HV], F32, tag="st")
                nc.sync.dma_start(st[:, 0:G3],
                                  w_hhT.ap()[128 * c:128 * (c + 1), :])
                nc.vector.tensor_copy(whh16[:, c, :], st[:, 0:G3])
            for c in range(NK):
                st = wst.tile([128, HV], F32, tag="st")
                nc.sync.dma_start(st[:, 0:G3],
                                  w_ih_eT.ap()[128 * c:128 * (c + 1), :])
                nc.vector.tensor_copy(wihe16[:, c, :], st[:, 0:G3])
            for c in range(NK):
                st = wst.tile([128, HV], F32, tag="st")
                nc.sync.dma_start(st[:, 0:TB],
                                  xsT.ap()[128 * c:128 * (c + 1), :])
                nc.vector.tensor_copy(
                    xs16[:, c, :, :],
                    st[:, 0:TB].rearrange("p (t b) -> p t b", b=B))

        # ---------------- phase E: recurrence + projection ----------------
        psG = ctx.enter_context(tc.tile_pool(name="psG", bufs=1, space="PSUM"))
        psT = ctx.enter_context(tc.tile_pool(name="psT", bufs=1, space="PSUM"))
        psP = ctx.enter_context(tc.tile_pool(name="psP", bufs=3, space="PSUM"))

        nc.vector.memset(hT16_0[:], 0.0)
        hT16 = hT16_0
        h_prev = hpool.tile([64, H], F32, tag="h")
        nc.vector.memset(h_prev[:], 0.0)

        def emit_proj(g):
            nstep = min(2, NT - 2 * g)
            m = B * nstep
            ts = slice(2 * g, 2 * g + nstep)
            for wave in WAVES:
                pss = {}
                for vt in wave:
                    pss[vt] = psP.tile([128, VT], F32, tag="lg",
                                       name=f"lg_{g}_{vt}")
                for c in range(NK):
                    stat = hTh[:, c, ts, :]
                    for vt in wave:
                        nc.tensor.matmul(
                            pss[vt][0:m, :], stat,
                            wh[:, c, VT * vt:VT * (vt + 1)],
                            start=(c == 0), stop=False)
                    for vt in wave:
                        nc.tensor.matmul(
                            pss[vt][0:m, :], stat,
                            wl[:, c, VT * vt:VT * (vt + 1)],
                            start=False, stop=False)
                for c in range(NK):
                    stat = hTl[:, c, ts, :]
                    for vt in wave:
                        nc.tensor.matmul(
                            pss[vt][0:m, :], stat,
                            wh[:, c, VT * vt:VT * (vt + 1)],
                            start=False, stop=(c == NK - 1))
                for vt in wave:
                    lg = evac.tile([128, VT], F32, tag="ev")
                    nc.vector.tensor_copy(lg[0:m, :], pss[vt][0:m, :])
                    nc.sync.dma_start(
                        logits.ap()[128 * g:128 * g + m,
                                    VT * vt:VT * (vt + 1)],
                        lg[0:m, :])

        for t in range(NT):
            # G = [h;x_t] @ [W_hh; W_ih_emb]^T, fp16 operands, f32 psum.
            # layout: [0:1024]=r|z fused, [1024:1536]=gh_n, [1536:2048]=gi_n
            G = psG.tile([64, 2048], F32, tag="G")
            for idx in range(2 * NK):
                if idx < NK:
                    c = idx
                    st, wm = hT16[:, c, :], whh16
                else:
                    c = idx - NK
                    st, wm = xs16[:, c, t, :], wihe16
                nc.tensor.matmul(G[:, 0:512], st, wm[:, c, 0:512],
                                 start=(idx == 0), stop=(idx == 2 * NK - 1))
                nc.tensor.matmul(G[:, 512:1024], st, wm[:, c, 512:1024],
                                 start=(idx == 0), stop=(idx == 2 * NK - 1))
                if idx < NK:
                    nc.tensor.matmul(G[:, 1024:1536], st, wm[:, c, 1024:1536],
                                     start=(idx == 0), stop=(idx == NK - 1))
                else:
                    nc.tensor.matmul(G[:, 1536:2048], st, wm[:, c, 1024:1536],
                                     start=(idx == NK),
                                     stop=(idx == 2 * NK - 1))

            s_rz = gates.tile([64, 2 * H], F32, tag="srz")
            nc.vector.tensor_add(s_rz[:], G[:, 0:2 * H], gicb[:, 0:2 * H])
            rz = gates.tile([64, 2 * H], F32, tag="rz")
            nc.scalar.activation(rz[:], s_rz[:], AF.Sigmoid)
            ghn = gates.tile([64, H], F32, tag="ghn")
            nc.vector.tensor_add(ghn[:], G[:, 2 * H:3 * H], bhh_n[:])
            gin = gates.tile([64, H], F32, tag="gin")
            nc.vector.tensor_add(gin[:], G[:, 3 * H:4 * H], gicb[:, 2 * H:])
            tn = gates.tile([64, H], F32, tag="tn")
            nc.vector.tensor_mul(tn[:], rz[:, 0:H], ghn[:])
            tn2 = gates.tile([64, H], F32, tag="tn2")
            nc.vector.tensor_add(tn2[:], tn[:], gin[:])
            n = gates.tile([64, H], F32, tag="n")
            nc.scalar.activation(n[:], tn2[:], AF.Tanh)
            hmn = gates.tile([64, H], F32, tag="hmn")
            nc.vector.tensor_sub(hmn[:], h_prev[:], n[:])
            zh = gates.tile([64, H], F32, tag="zh")
            nc.vector.tensor_mul(zh[:], rz[:, H:2 * H], hmn[:])
            h_new = hpool.tile([64, H], F32, tag="h")
            nc.vector.tensor_add(h_new[:], n[:], zh[:])
            h_prev = h_new

            # transpose h (f32) -> [H,B]; derive fp16 (next step) + bf16 hi/lo
            hT = psT.tile([128, NK, B], F32, tag="hT")
            for c in range(NK):
                nc.tensor.transpose(hT[:, c, :],
                                    h_new[:, 128 * c:128 * (c + 1)],
                                    ident[0:64, 0:64])
            hT16 = htp.tile([128, NK, B], F16, tag="ht16")
            nc.vector.tensor_copy(hT16[:], hT[:])
            nc.vector.tensor_copy(hTh[:, :, t, :], hT[:])
            nc.vector.tensor_sub(hTl[:, :, t, :], hT[:], hTh[:, :, t, :])

            if t % 2 == 1:
                emit_proj(t // 2)
        if NT % 2 == 1:
            emit_proj(NGRP - 1)


_CACHE = {}


def _build():
    if "nc" in _CACHE:
        return _CACHE["nc"]
    nc = bacc.Bacc("TRN2", debug=False)
    dram = (
        nc.dram_tensor("encT", [H, BS], F32, kind="ExternalInput"),
        nc.dram_tensor("xsT", [E, TB], F32, kind="ExternalInput"),
        nc.dram_tensor("w_hhT", [H, G3], F32, kind="ExternalInput"),
        nc.dram_tensor("w_ih_eT", [E, G3], F32, kind="ExternalInput"),
        nc.dram_tensor("w_ih_cT", [H, G3], F32, kind="ExternalInput"),
        nc.dram_tensor("w1e", [H, H], F32, kind="ExternalInput"),
        nc.dram_tensor("w2", [H, H], F32, kind="ExternalInput"),
        nc.dram_tensor("w3", [H, H], F32, kind="ExternalInput"),
        nc.dram_tensor("v4", [NK, 128], F32, kind="ExternalInput"),
        nc.dram_tensor("b_ih", [1, G3], F32, kind="ExternalInput"),
        nc.dram_tensor("b_hh", [1, G3], F32, kind="ExternalInput"),
        nc.dram_tensor("w_outT", [H, VL], F32, kind="ExternalInput"),
        nc.dram_tensor("logits", [TB, VL], F32, kind="ExternalOutput"),
    )
    with TileContext(nc) as tc:
        _emit(nc, tc, dram)
    nc.compile()
    _CACHE["nc"] = nc
    return nc


def kernel(**inputs):
    f = lambda k: np.asarray(inputs[k], np.float32)
    enc = f("encoder_output")
    tgt = np.asarray(inputs["targets"])
    emb = f("embedding")
    W1 = f("W1")
    W2 = f("W2")
    W3 = f("W3")
    v_att = f("v_att")
    W_ih, b_ih = f("W_ih"), f("b_ih")
    W_hh, b_hh = f("W_hh"), f("b_hh")
    W_out, b_out = f("W_out"), f("b_out")

    C = np.ascontiguousarray
    xs = emb[tgt[:, :NT]]                                   # [B,NT,E] gather
    common = {
        "encT": C(enc.transpose(2, 0, 1).reshape(H, BS)),
        "xsT": C(xs.transpose(2, 1, 0).reshape(E, TB)),     # [E][t][b]
        "w_hhT": C(W_hh.T),
        "w_ih_eT": C(W_ih[:, :E].T),
        "w_ih_cT": C(W_ih[:, E:].T),
        "w1e": C(W1[:, :H]),
        "w2": C(W2),
        "w3": C(W3),
        "v4": C(v_att.reshape(NK, 128)),
        "b_ih": C(b_ih.reshape(1, G3)),
        "b_hh": C(b_hh.reshape(1, G3)),
    }
    in_maps = []
    for c in range(NCORE):
        m = dict(common)
        m["w_outT"] = C(W_out[VL * c:VL * (c + 1), :].T)
        in_maps.append(m)

    nc = _build()
    res = run_bass_kernel_spmd(nc, in_maps, core_ids=list(range(NCORE)))

    parts = [res.results[c]["logits"] for c in range(NCORE)]
    full = np.concatenate(parts, axis=1).reshape(NT, B, V).transpose(1, 0, 2)
    full = np.ascontiguousarray(full)
    if b_out.any():
        full += b_out[None, None, :]
    preds = full.argmax(axis=2).astype(np.int32)
    return full, preds


if __name__ == "__main__":
    import reference as R
    inp = {k: np.asarray(v) for k, v in R.setup_inputs().items()}
    lg, pd = kernel(**inp)
    print("out", lg.shape, lg.dtype, pd.shape, pd.dtype)


# revision 5
# speedup vs baseline: 171.8330x; 171.8330x over previous
"""Trainium2 Bass kernel for nn_DecoderRNN (Bahdanau-attention GRU decoder).

Key algebraic simplification (verified exact vs the reference): the attention
MLP has no nonlinearities, so the pre-softmax score is
    score[b,s] = enc[b,s,:].(W1_enc^T u) + (h-dependent terms constant in s),
with u = W2^T W3^T v_att.  Softmax over s is shift-invariant, so the attention
weights are independent of the hidden state and of t.  The whole attention
collapses to a one-time precompute of a per-batch context vector; the rest is
one GRU recurrence plus one large output projection.

Distribution across the 8 cores: the recurrence (small, weight-streaming
bound, doesn't shard) is replicated; the vocab projection W_out (the dominant
cost) is sharded column-wise, 4000 vocab columns per core.  Full inputs are
taken on the host, per-core input maps are built, and the 8 logits slices are
gathered and reassembled.

Precision (chosen so that argmax(logits) matches an fp32 reference exactly on
this problem's data): fp16 matmuls for the recurrence and embedding input
path, fp32 for softmax/context/gates, and a 3-pass bf16 hi/lo split
(bf16x3 pseudo-fp32) for the vocab projection.
"""

import sys

sys.path.insert(0, "/opt/trn_rl_repo")

from contextlib import ExitStack

import numpy as np

import concourse.bacc as bacc
import concourse.mybir as mybir
from concourse import masks
from concourse.tile import TileContext
from concourse.bass_utils import run_bass_kernel_spmd

F32 = mybir.dt.float32
F16 = mybir.dt.float16
BF16 = mybir.dt.bfloat16
AF = mybir.ActivationFunctionType
OP = mybir.AluOpType
AX = mybir.AxisListType

B, S, H, E, T, V = 64, 80, 512, 512, 30, 32000
NCORE = 8
VL = V // NCORE            # 4000 vocab cols per core
NT = T - 1                 # 29 decode steps
TB = NT * B                # 1856 (t-major row index)
NK = H // 128              # 4 contraction chunks over H (= over E)
G3 = 3 * H                 # 1536 gate dim
NGRP = (NT + 1) // 2       # 15 projection groups of <=2 steps
VT = 500                   # vocab tile (one psum bank holds 512 f32)
NVT = VL // VT             # 8
WAVES = [(0, 1, 2), (3, 4, 5), (6, 7)]
BS = B * S


def _emit(nc, tc, dram):
    (encT, xsT, w_hhT, w_ih_eT, w_ih_cT, w1e, w2, w3, v4, b_ih, b_hh,
     w_outT, logits) = dram

    ctx = ExitStack()
    with ctx:
        pers = ctx.enter_context(tc.tile_pool(name="pers", bufs=1))
        dscr = ctx.enter_context(tc.tile_pool(name="dscr", bufs=1,
                                              space="DRAM"))

        ident = pers.tile([128, 128], F32)
        masks.make_identity(nc, ident[:])
        ones = pers.tile([128, 1], F32)
        nc.vector.memset(ones[:], 1.0)

        gicb = pers.tile([64, G3], F32)      # gi_ctx + b_ih (+ b_hh on r,z)
        bhh_n = pers.tile([64, H], F32)      # b_hh n-part broadcast
        whh16 = pers.tile([128, NK, G3], F16)
        wihe16 = pers.tile([128, NK, G3], F16)
        xs16 = pers.tile([128, NK, NT, B], F16)
        hTh = pers.tile([128, NK, NT, B], BF16)
        hTl = pers.tile([128, NK, NT, B], BF16)
        ctxT = pers.tile([128, NK, B], F32)
        wvec = pers.tile([128, NK], F32)
        hT16_0 = pers.tile([128, NK, B], F16)

        e_scr = dscr.tile([1, BS], F32)
        rs_scr = dscr.tile([1, B], F32)

        # ---------------- phase A/B: attention precompute ----------------
        with tc.tile_pool(name="p0", bufs=1) as p0, \
             tc.tile_pool(name="p0s", bufs=2) as p0s, \
             tc.tile_pool(name="p0e", bufs=2) as p0e, \
             tc.tile_pool(name="ps0", bufs=1, space="PSUM") as ps0:
            # v_att -> [128, 4] (chunk-major)
            vsb = p0.tile([128, NK], F32)
            nc.sync.dma_start(vsb[:], v4.ap().rearrange("c p -> p c"))

            # matvec chain w = W1_enc^T (W2^T (W3^T v_att)), fp32 on PE
            src = vsb
            for wdram, last in ((w3, False), (w2, False), (w1e, True)):
                wmat = p0s.tile([128, NK, H], F32, tag="wmat")
                for c in range(NK):
                    nc.sync.dma_start(
                        wmat[:, c, :], wdram.ap()[128 * c:128 * (c + 1), :])
                mv = ps0.tile([128, NK], F32, tag="mv")
                for j in range(NK):
                    for c in range(NK):
                        nc.tensor.matmul(
                            mv[:, j:j + 1], wmat[:, c, 128 * j:128 * (j + 1)],
                            src[:, c:c + 1], start=(c == 0), stop=(c == NK - 1))
                dst = wvec if last else p0s.tile([128, NK], F32, tag="mvout")
                nc.vector.tensor_copy(dst[:], mv[:])
                src = dst

            # scores[b,s] = sum_f encT[f,bs] w[f]: DVE per-chunk partials in
            # acc, then PE ones-matmul partition-reduce; exp on ACT; e goes to
            # DRAM so it can be broadcast-DMA'd back across partitions.
            acc = p0.tile([128, BS], F32)
            for c in range(NK):
                encc = p0e.tile([128, BS], F32, tag="encc")
                nc.sync.dma_start(encc[:], encT.ap()[128 * c:128 * (c + 1), :])
                if c == 0:
                    nc.vector.tensor_scalar_mul(acc[:], encc[:], wvec[:, 0:1])
                else:
                    nc.vector.scalar_tensor_tensor(
                        acc[:], encc[:], wvec[:, c:c + 1], acc[:],
                        op0=OP.mult, op1=OP.add)
            for j in range(BS // 512):
                sc = ps0.tile([1, 512], F32, tag="sc")
                nc.tensor.matmul(sc[:], ones[:], acc[:, 512 * j:512 * (j + 1)],
                                 start=True, stop=True)
                ej = p0s.tile([1, 512], F32, tag="ej")
                nc.scalar.activation(ej[:], sc[:], AF.Exp)
                nc.sync.dma_start(e_scr[:, 512 * j:512 * (j + 1)], ej[:])
            ebc = p0.tile([128, BS], F32)
            nc.gpsimd.dma_start(ebc[:], e_scr[:].broadcast_to([128, BS]))

            # per-batch 1/sum_s e, broadcast across partitions the same way
            rs = p0s.tile([1, B], F32, tag="rs")
            nc.vector.tensor_reduce(
                rs[:], ebc[0:1, :].rearrange("p (b s) -> p b s", s=S),
                axis=AX.X, op=OP.add)
            rsr = p0s.tile([1, B], F32, tag="rsr")
            nc.vector.reciprocal(rsr[:], rs[:])
            nc.sync.dma_start(rs_scr[:], rsr[:])
            rsb = p0.tile([128, B], F32)
            nc.gpsimd.dma_start(rsb[:], rs_scr[:].broadcast_to([128, B]))

            # context^T[f, b] = (sum_s e[b,s] enc[b,s,f]) / rs[b]
            for c in range(NK):
                encc = p0e.tile([128, BS], F32, tag="encc")
                nc.sync.dma_start(encc[:], encT.ap()[128 * c:128 * (c + 1), :])
                nc.vector.tensor_mul(acc[:], encc[:], ebc[:])
                ctxu = p0s.tile([128, B], F32, tag="ctxu")
                nc.vector.tensor_reduce(
                    ctxu[:], acc[:].rearrange("p (b s) -> p b s", s=S),
                    axis=AX.X, op=OP.add)
                nc.vector.tensor_mul(ctxT[:, c, :], ctxu[:], rsb[:])

        # ---------------- phase C: gi_ctx + bias folding ----------------
        with tc.tile_pool(name="p1", bufs=1) as p1, \
             tc.tile_pool(name="ps1", bufs=1, space="PSUM") as ps1:
            wihc = p1.tile([128, NK, G3], F32)
            for c in range(NK):
                nc.sync.dma_start(
                    wihc[:, c, :], w_ih_cT.ap()[128 * c:128 * (c + 1), :])
            gic = ps1.tile([64, G3], F32)
            for c in range(NK):
                for nn in range(3):
                    nc.tensor.matmul(
                        gic[:, 512 * nn:512 * (nn + 1)], ctxT[:, c, :],
                        wihc[:, c, 512 * nn:512 * (nn + 1)],
                        start=(c == 0), stop=(c == NK - 1))
            bihbc = p1.tile([64, G3], F32)
            nc.gpsimd.dma_start(bihbc[:], b_ih.ap().broadcast_to([64, G3]))
            bhhbc = p1.tile([64, G3], F32)
            nc.gpsimd.dma_start(bhhbc[:], b_hh.ap().broadcast_to([64, G3]))
            brz = p1.tile([64, 2 * H], F32)
            nc.vector.tensor_add(brz[:], bihbc[:, 0:2 * H], bhhbc[:, 0:2 * H])
            nc.vector.tensor_add(gicb[:, 0:2 * H], gic[:, 0:2 * H], brz[:])
            nc.vector.tensor_add(gicb[:, 2 * H:], gic[:, 2 * H:],
                                 bihbc[:, 2 * H:])
            nc.vector.tensor_copy(bhh_n[:], bhhbc[:, 2 * H:])

        # ---------------- phase D: weight conversion --------------------
        hpool = ctx.enter_context(tc.tile_pool(name="h", bufs=2))
        gates = ctx.enter_context(tc.tile_pool(name="gates", bufs=1))
        htp = ctx.enter_context(tc.tile_pool(name="htp", bufs=2))
        evac = ctx.enter_context(tc.tile_pool(name="evac", bufs=4))
        wout = ctx.enter_context(tc.tile_pool(name="wout", bufs=1))
        wh = wout.tile([128, NK, VL], BF16)
        wl = wout.tile([128, NK, VL], BF16)
        with tc.tile_pool(name="wst", bufs=2) as wst:
            HV = VL // 2
            for c in range(NK):
                st = wst.tile([128, HV], F32, tag="st")
                nc.sync.dma_start(st[:, 0:G3],
                                  w_hhT.ap()[128 * c:128 * (c + 1), :])
                nc.vector.tensor_copy(whh16[:, c, :], st[:, 0:G3])
            for c in range(NK):
                st = wst.tile([128, HV], F32, tag="st")
                nc.sync.dma_start(st[:, 0:G3],
                                  w_ih_eT.ap()[128 * c:128 * (c + 1), :])
                nc.vector.tensor_copy(wihe16[:, c, :], st[:, 0:G3])
            for c in range(NK):
                st = wst.tile([128, HV], F32, tag="st")
                nc.sync.dma_start(st[:, 0:TB],
                                  xsT.ap()[128 * c:128 * (c + 1), :])
                nc.vector.tensor_copy(
                    xs16[:, c, :, :],
                    st[:, 0:TB].rearrange("p (t b) -> p t b", b=B))
            for c in range(NK):
                for hh in range(2):
                    st = wst.tile([128, HV], F32, tag="st")
                    sl = slice(HV * hh, HV * (hh + 1))
                    nc.sync.dma_start(
                        st[:], w_outT.ap()[128 * c:128 * (c + 1), sl])
                    nc.vector.tensor_copy(wh[:, c, sl], st[:])
                    nc.vector.tensor_sub(wl[:, c, sl], st[:], wh[:, c, sl])

        # ---------------- phase E: recurrence + projection ----------------
        psG = ctx.enter_context(tc.tile_pool(name="psG", bufs=1, space="PSUM"))
        psT = ctx.enter_context(tc.tile_pool(name="psT", bufs=1, space="PSUM"))
        psP = ctx.enter_context(tc.tile_pool(name="psP", bufs=3, space="PSUM"))

        nc.vector.memset(hT16_0[:], 0.0)
        hT16 = hT16_0
        h_prev = hpool.tile([64, H], F32, tag="h")
        nc.vector.memset(h_prev[:], 0.0)

        def emit_proj(g):
            nstep = min(2, NT - 2 * g)
            m = B * nstep
            ts = slice(2 * g, 2 * g + nstep)
            for wave in WAVES:
                pss = {}
                for vt in wave:
                    pss[vt] = psP.tile([128, VT], F32, tag="lg",
                                       name=f"lg_{g}_{vt}")
                for c in range(NK):
                    stat = hTh[:, c, ts, :]
                    for vt in wave:
                        nc.tensor.matmul(
                            pss[vt][0:m, :], stat,
                            wh[:, c, VT * vt:VT * (vt + 1)],
                            start=(c == 0), stop=False)
                    for vt in wave:
                        nc.tensor.matmul(
                            pss[vt][0:m, :], stat,
                            wl[:, c, VT * vt:VT * (vt + 1)],
                            start=False, stop=False)
                for c in range(NK):
                    stat = hTl[:, c, ts, :]
                    for vt in wave:
                        nc.tensor.matmul(
                            pss[vt][0:m, :], stat,
                            wh[:, c, VT * vt:VT * (vt + 1)],
                            start=False, stop=(c == NK - 1))
                for vt in wave:
                    lg = evac.tile([128, VT], F32, tag="ev")
                    nc.vector.tensor_copy(lg[0:m, :], pss[vt][0:m, :])
                    nc.sync.dma_start(
                        logits.ap()[128 * g:128 * g + m,
                                    VT * vt:VT * (vt + 1)],
                        lg[0:m, :])

        for t in range(NT):
            # G = [h;x_t] @ [W_hh; W_ih_emb]^T, fp16 operands, f32 psum.
            # layout: [0:1024]=r|z fused, [1024:1536]=gh_n, [1536:2048]=gi_n
            G = psG.tile([64, 2048], F32, tag="G")
            for idx in range(2 * NK):
                if idx < NK:
                    c = idx
                    st, wm = hT16[:, c, :], whh16
                else:
                    c = idx - NK
                    st, wm = xs16[:, c, t, :], wihe16
                nc.tensor.matmul(G[:, 0:512], st, wm[:, c, 0:512],
                                 start=(idx == 0), stop=(idx == 2 * NK - 1))
                nc.tensor.matmul(G[:, 512:1024], st, wm[:, c, 512:1024],
                                 start=(idx == 0), stop=(idx == 2 * NK - 1))
                if idx < NK:
                    nc.tensor.matmul(G[:, 1024:1536], st, wm[:, c, 1024:1536],
                                     start=(idx == 0), stop=(idx == NK - 1))
                else:
                    nc.tensor.matmul(G[:, 1536:2048], st, wm[:, c, 1024:1536],
                                     start=(idx == NK),
                                     stop=(idx == 2 * NK - 1))

            s_rz = gates.tile([64, 2 * H], F32, tag="srz")
            nc.vector.tensor_add(s_rz[:], G[:, 0:2 * H], gicb[:, 0:2 * H])
            rz = gates.tile([64, 2 * H], F32, tag="rz")
            nc.scalar.activation(rz[:], s_rz[:], AF.Sigmoid)
            ghn = gates.tile([64, H], F32, tag="ghn")
            nc.vector.tensor_add(ghn[:], G[:, 2 * H:3 * H], bhh_n[:])
            gin = gates.tile([64, H], F32, tag="gin")
            nc.vector.tensor_add(gin[:], G[:, 3 * H:4 * H], gicb[:, 2 * H:])
            tn = gates.tile([64, H], F32, tag="tn")
            nc.vector.tensor_mul(tn[:], rz[:, 0:H], ghn[:])
            tn2 = gates.tile([64, H], F32, tag="tn2")
            nc.vector.tensor_add(tn2[:], tn[:], gin[:])
            n = gates.tile([64, H], F32, tag="n")
            nc.scalar.activation(n[:], tn2[:], AF.Tanh)
            hmn = gates.tile([64, H], F32, tag="hmn")
            nc.vector.tensor_sub(hmn[:], h_prev[:], n[:])
            zh = gates.tile([64, H], F32, tag="zh")
            nc.vector.tensor_mul(zh[:], rz[:, H:2 * H], hmn[:])
            h_new = hpool.tile([64, H], F32, tag="h")
            nc.vector.tensor_add(h_new[:], n[:], zh[:])
            h_prev = h_new

            # transpose h (f32) -> [H,B]; derive fp16 (next step) + bf16 hi/lo
            hT = psT.tile([128, NK, B], F32, tag="hT")
            for c in range(NK):
                nc.tensor.transpose(hT[:, c, :],
                                    h_new[:, 128 * c:128 * (c + 1)],
                                    ident[0:64, 0:64])
            hT16 = htp.tile([128, NK, B], F16, tag="ht16")
            nc.vector.tensor_copy(hT16[:], hT[:])
            nc.vector.tensor_copy(hTh[:, :, t, :], hT[:])
            nc.vector.tensor_sub(hTl[:, :, t, :], hT[:], hTh[:, :, t, :])

            if t % 2 == 1 and t // 2 >= 1:
                emit_proj(t // 2 - 1)
        emit_proj(NGRP - 2)
        emit_proj(NGRP - 1)


_CACHE = {}


def _build():
    if "nc" in _CACHE:
        return _CACHE["nc"]
    nc = bacc.Bacc("TRN2", debug=False)
    dram = (
        nc.dram_tensor("encT", [H, BS], F32, kind="ExternalInput"),
        nc.dram_tensor("xsT", [E, TB], F32, kind="ExternalInput"),
        nc.dram_tensor("w_hhT", [H, G3], F32, kind="ExternalInput"),
        nc.dram_tensor("w_ih_eT", [E, G3], F32, kind="ExternalInput"),
        nc.dram_tensor("w_ih_cT", [H, G3], F32, kind="ExternalInput"),
        nc.dram_tensor("w1e", [H, H], F32, kind="ExternalInput"),
        nc.dram_tensor("w2", [H, H], F32, kind="ExternalInput"),
        nc.dram_tensor("w3", [H, H], F32, kind="ExternalInput"),
        nc.dram_tensor("v4", [NK, 128], F32, kind="ExternalInput"),
        nc.dram_tensor("b_ih", [1, G3], F32, kind="ExternalInput"),
        nc.dram_tensor("b_hh", [1, G3], F32, kind="ExternalInput"),
        nc.dram_tensor("w_outT", [H, VL], F32, kind="ExternalInput"),
        nc.dram_tensor("logits", [TB, VL], F32, kind="ExternalOutput"),
    )
    with TileContext(nc) as tc:
        _emit(nc, tc, dram)
    nc.compile()
    _CACHE["nc"] = nc
    return nc


def kernel(**inputs):
    f = lambda k: np.asarray(inputs[k], np.float32)
    enc = f("encoder_output")
    tgt = np.asarray(inputs["targets"])
    emb = f("embedding")
    W1 = f("W1")
    W2 = f("W2")
    W3 = f("W3")
    v_att = f("v_att")
    W_ih, b_ih = f("W_ih"), f("b_ih")
    W_hh, b_hh = f("W_hh"), f("b_hh")
    W_out, b_out = f("W_out"), f("b_out")

    C = np.ascontiguousarray
    xs = emb[tgt[:, :NT]]                                   # [B,NT,E] gather
    common = {
        "encT": C(enc.transpose(2, 0, 1).reshape(H, BS)),
        "xsT": C(xs.transpose(2, 1, 0).reshape(E, TB)),     # [E][t][b]
        "w_hhT": C(W_hh.T),
        "w_ih_eT": C(W_ih[:, :E].T),
        "w_ih_cT": C(W_ih[:, E:].T),
        "w1e": C(W1[:, :H]),
        "w2": C(W2),
        "w3": C(W3),
        "v4": C(v_att.reshape(NK, 128)),
        "b_ih": C(b_ih.reshape(1, G3)),
        "b_hh": C(b_hh.reshape(1, G3)),
    }
    in_maps = []
    for c in range(NCORE):
        m = dict(common)
        m["w_outT"] = C(W_out[VL * c:VL * (c + 1), :].T)
        in_maps.append(m)

    nc = _build()
    res = run_bass_kernel_spmd(nc, in_maps, core_ids=list(range(NCORE)))

    parts = [res.results[c]["logits"] for c in range(NCORE)]
    full = np.concatenate(parts, axis=1).reshape(NT, B, V).transpose(1, 0, 2)
    full = np.ascontiguousarray(full)
    if b_out.any():
        full += b_out[None, None, :]
    preds = full.argmax(axis=2).astype(np.int32)
    return full, preds


if __name__ == "__main__":
    import reference as R
    inp = {k: np.asarray(v) for k, v in R.setup_inputs().items()}
    lg, pd = kernel(**inp)
    print("out", lg.shape, lg.dtype, pd.shape, pd.dtype)
